# revision 25
# baseline (speedup 1.0000x reference)
"""CLIPtrase recalibration kernel for Trainium2 (Bass/Tile), fp8 DoubleRow.

Per-batch computation (B=8, one batch element per NeuronCore):
    x  : (2304, 768) f32
    xn = x / ||x||_row
    S  = xn @ xn.T              (cosine correlation, symmetric)
    W  = softmax(S / 0.05, axis=-1)
    out = 0.5 * x + 0.5 * (W @ x)

Key ideas vs the bf16 baseline:
  - All matmuls run in fp8e4 (e4m3) with MatmulPerfMode.DoubleRow: each
    instruction contracts TWO 128-deep k-tiles (157 TF/s peak, 2x bf16).
  - Only normalized xn is kept in fp8. PV needs E @ x = (n_a * E) @ xn, so
    ln(n_a) is folded into the exp bias (per-partition bias AP):
    E2^T[a,q] = exp(20*S^T + (ln n_a - 20)). The softmax denominator comes
    from an aug column of xn holding 1/n_a.
  - Exact self-term: the diagonal of S is killed in PSUM by accumulating a
    -240*I matmul before exp, so E2 has zero diagonal. The true diagonal
    weight is exp(0) = 1 exactly, re-added in f32 at blend time:
        recal = (x + num') / (1 + den')
    so fp8 quantization never touches the dominant self-term.
  - ACT uses only Ln/Exp/Copy — all in the natural_log_exp_and_others
    table, so the act table is loaded once (no ACT_TABLE_LOAD thrash).
    1/n = Exp(-0.5*Ln(ssum)); ln n - 20 = Copy(0.5*Ln(ssum) - 20).
  - Engine balance: ACT = exps + blend muls; DVE = ssum + PSUM->SBUF
    copies + blend scalars; GPSIMD = xn casts + blend fma (SBUF-only ops);
    PE is the critical path (~95us of fp8 matmul + transposes).
  - E^T blocks for the lower triangle are PE transposes of stored upper
    blocks (fp8 transpose writes PSUM at element step 2), staged to SBUF
    by DVE, interleaved between PV pairs to keep PE fed.
  - Output stored bf16 (halves output DMA); host casts back to f32.
"""

import sys

sys.path.insert(0, "/opt/trn_rl_repo")

import numpy as np

import concourse.bass as bass
import concourse.mybir as mybir
from concourse import bacc
from concourse.bass_utils import run_bass_kernel_spmd
from concourse.masks import make_identity
from concourse.tile import TileContext

F32 = mybir.dt.float32
BF16 = mybir.dt.bfloat16
F8 = mybir.dt.float8e4
DR = mybir.MatmulPerfMode.DoubleRow
AF = mybir.ActivationFunctionType

B = 8
H = 48
W = 48
N = H * W          # 2304
D = 768
P = 128
NT = N // P        # 18 row tiles
DT = D // P        # 6 feature tiles
DA = D + 4         # aug width: col 768 = 1/n, cols 769..771 = 0 pad
TEMP_INV = 20.0    # 1 / 0.05
# query megablocks: (start, width)
MEGAS = [(0, 512), (512, 512), (1024, 512), (1536, 512), (2048, 256)]
# PV strips within the psum tile; avoid crossing the 512-col bank boundary
PV_STRIPS = [(0, 256), (256, 256), (512, 256), (768, 4)]
NG = 4             # norm batch: tiles per DVE norm group (aligns megas)
# DVE bit-trick constants: ln n - 20 = 0.5*ln(ssum) - 20 via the float
# bit pattern (log2(s) ~ i/2^23 - 126.955), rsqrt via 0x5f3759df + Newton
LN_C1 = 4.1314791474339085e-08
LN_C2 = -63.99925015399393
RSQRT_MAGIC = 1597463007
I32 = mybir.dt.int32

_CACHED = {}


def build_program():
    nc = bacc.Bacc()
    x_in = nc.declare_dram_parameter("x", [N, D], F32, isOutput=False)
    out_dram = nc.declare_dram_parameter("out", [N, D], BF16, isOutput=True)

    mega_of_block = {}
    amax = []
    for mi, (q0, qw) in enumerate(MEGAS):
        for b in range(q0 // P, (q0 + qw) // P):
            mega_of_block[b] = mi
        amax.append((q0 + qw) // P - 1)

    with TileContext(nc) as tc:
        with (
            tc.tile_pool(name="persist", bufs=1) as persist,
            tc.tile_pool(name="work", bufs=3) as work,
            tc.tile_pool(name="stg", bufs=10) as stgp,
            tc.tile_pool(name="estore", bufs=1) as estore,
            tc.tile_pool(name="psS", bufs=2, space="PSUM") as psum_s,
            tc.tile_pool(name="psPV", bufs=2, space="PSUM") as psum_pv,
            tc.tile_pool(name="psT", bufs=2, space="PSUM") as psum_t,
        ):
            # persistent tensors
            x_full = persist.tile([P, NT, D], F32)     # raw x, resident
            xT = persist.tile([P, DT, N], F8)          # xn^T  [d, n]
            xn_aug = persist.tile([P, NT, DA], F8)     # [xn | 1/n | 0 0 0]
            ident = persist.tile([P, P], F8)
            make_identity(nc, ident)
            negdiag = persist.tile([P, P], F8)         # -240 * I
            nc.gpsimd.memset(negdiag, 0.0)
            nc.gpsimd.tensor_scalar_mul(negdiag, ident, -240.0)
            nc.vector.memset(xn_aug[:, :, D + 1 : DA], 0.0)
            zero_bias = persist.tile([P, 1], F32)
            nc.vector.memset(zero_bias, 0.0)
            lnb = persist.tile([P, NT], F32)           # ln(n) - 20 per tile
            invn = persist.tile([P, NT], F32)          # 1/n per tile
            ssums = persist.tile([P, NT], F32)         # sum(x^2) per tile

            # E2 storage: per mega m, E2^T[a, q-cols] for a <= amax(m)
            e_tiles = [
                estore.tile(
                    [P, amax[mi] + 1, qw], F8, tag=f"E{mi}", name=f"E{mi}"
                )
                for mi, (q0, qw) in enumerate(MEGAS)
            ]

            # ---- Phase 1+2 fused: load, norms, xn, xT; then per norm
            # group emit the QK mega whose tiles just completed ----
            # input DMAs issued up front, alternating issue queues so the
            # per-queue ~600ns issue serialization halves
            for j in range(NT):
                eng = nc.sync if j % 2 == 0 else nc.gpsimd
                if j < 4:
                    # split early tiles so the first norm chain starts sooner
                    hd = D // 2
                    for c in range(2):
                        eng.dma_start(
                            out=x_full[:, j, c * hd : (c + 1) * hd],
                            in_=x_in[j * P : (j + 1) * P,
                                     c * hd : (c + 1) * hd],
                        )
                else:
                    eng.dma_start(
                        out=x_full[:, j, :],
                        in_=x_in[j * P : (j + 1) * P, :],
                    )

            def emit_norm_group(g0, g1):
                """Norms for tiles [g0, g1) entirely on DVE via float bit
                tricks (no ACT Ln/Sqrt -> single ACT table for the kernel).
                lnb = 0.5*ln(ssum) - 20 from the exponent bits;
                invn = rsqrt(ssum) via 0x5f3759df + one Newton step."""
                k = g1 - g0
                sl = slice(g0, g1)
                ibits = ssums[:, sl].bitcast(I32)
                fi = work.tile([P, k], F32, tag="nfi")
                nc.vector.tensor_copy(fi, ibits)  # int32 -> f32 numeric
                nc.vector.tensor_scalar(
                    lnb[:, sl], fi, LN_C1, LN_C2,
                    op0=mybir.AluOpType.mult, op1=mybir.AluOpType.add,
                )
                ish = work.tile([P, k], I32, tag="nish")
                nc.vector.tensor_scalar(
                    ish, ibits, 1, None,
                    op0=mybir.AluOpType.logical_shift_right,
                )
                iy = work.tile([P, k], I32, tag="niy")
                nc.vector.tensor_scalar(
                    iy, ish, -1, RSQRT_MAGIC,
                    op0=mybir.AluOpType.mult, op1=mybir.AluOpType.add,
                )
                y0 = iy.bitcast(F32)
                t1 = work.tile([P, k], F32, tag="nt1")
                nc.vector.tensor_mul(t1, y0, y0)
                t2 = work.tile([P, k], F32, tag="nt2")
                nc.vector.tensor_mul(t2, t1, ssums[:, sl])
                t3 = work.tile([P, k], F32, tag="nt3")
                nc.vector.tensor_scalar(
                    t3, t2, -0.5, 1.5,
                    op0=mybir.AluOpType.mult, op1=mybir.AluOpType.add,
                )
                nc.vector.tensor_mul(invn[:, sl], y0, t3)
                # aug cols: 1/n in fp8
                nc.vector.tensor_copy(xn_aug[:, sl, D : D + 1], invn[:, sl])

            def emit_tile(jj):
                # xn = x * (1/n), cast to fp8 (ACT Copy w/ scale AP)
                nc.scalar.mul(
                    xn_aug[:, jj, 0:D],
                    x_full[:, jj, :],
                    invn[:, jj : jj + 1],
                )
                # transpose xn into xT columns (fp8 transpose writes PSUM
                # at element step 2)
                ptj = psum_t.tile([P, DT, P, 2], F8, tag="pt")
                for d in range(DT):
                    nc.tensor.transpose(
                        ptj[:, d, :, 0],
                        xn_aug[:, jj, d * P : (d + 1) * P],
                        ident,
                    )
                nc.vector.tensor_copy(
                    xT[:, :, jj * P : (jj + 1) * P], ptj[:, :, :, 0]
                )

            def emit_qk_mega(mi):
                q0, qw = MEGAS[mi]
                et = e_tiles[mi]
                for a in range(amax[mi] + 1):
                    c0 = max(0, a * P - q0)
                    has_diag = a * P >= q0  # diag block at cols c0:c0+128
                    ps = psum_s.tile([P, 512], F32, tag="psS")
                    strips = []
                    s = c0
                    while s < qw:
                        w = min(256, qw - s)
                        strips.append((s, w))
                        s += w
                    for dp in range(DT // 2):
                        for si, (s, w) in enumerate(strips):
                            stop = dp == DT // 2 - 1 and not (
                                has_diag and si == 0
                            )
                            nc.tensor.matmul(
                                ps[:, s : s + w],
                                lhsT=xT[:, 2 * dp : 2 * dp + 2,
                                        a * P : (a + 1) * P],
                                rhs=xT[:, 2 * dp : 2 * dp + 2,
                                       q0 + s : q0 + s + w],
                                start=(dp == 0),
                                stop=stop,
                                perf_mode=DR,
                            )
                    if has_diag:
                        # kill diagonal: S += -240*I on cols c0:c0+128
                        nc.tensor.matmul(
                            ps[:, c0 : c0 + P],
                            lhsT=negdiag,
                            rhs=ident,
                            start=False,
                            stop=True,
                            skip_group_check=True,
                        )
                    # E2^T = exp(20*S + (ln n_a - 20)), fp8 out
                    nc.scalar.activation(
                        et[:, a, c0:qw],
                        ps[:, c0:qw],
                        AF.Exp,
                        bias=lnb[:, a : a + 1],
                        scale=TEMP_INV,
                    )

            # driver: norm groups align with megas (amax = 3,7,11,15,17);
            # mega mi is emitted as soon as its tiles complete. First two
            # groups are small to shorten the startup dependency chain.
            groups = [(0, 2, None), (2, 4, 0), (4, 8, 1), (8, 12, 2),
                      (12, 16, 3), (16, 18, 4)]
            for g0, g1, mi in groups:
                for j in range(g0, g1):
                    # sum of squares: ACT Square + accumulator
                    scratch = work.tile([P, D], BF16, tag="scratch")
                    nc.scalar.activation(
                        scratch,
                        x_full[:, j, :],
                        AF.Square,
                        bias=zero_bias,
                        accum_out=ssums[:, j : j + 1],
                    )
                emit_norm_group(g0, g1)
                for j in range(g0, g1):
                    emit_tile(j)
                if mi is not None:
                    emit_qk_mega(mi)

            def e_block(a, b):
                """AP of stored E2^T[a-tile, b-tile cols] (needs a <= b)."""
                mi = mega_of_block[b]
                q0 = MEGAS[mi][0]
                off = b * P - q0
                return e_tiles[mi][:, a, off : off + P]

            def e_pair(t, b):
                """AP of stored E2^T pair (2t,2t+1) x b-cols (2t+1 <= b)."""
                mi = mega_of_block[b]
                q0 = MEGAS[mi][0]
                off = b * P - q0
                return e_tiles[mi][:, 2 * t : 2 * t + 2, off : off + P]

            # ---- Phase 3: PV (fp8 DR) + staging + blend, pipelined ----
            def stage_groups(b):
                """Group list for b: each group is <=4 consecutive k-tiles
                to stage (transpose or copy) into one stg tile."""
                first = b if b % 2 == 0 else b + 1
                alist = list(range(first, NT))
                return [alist[i : i + 4] for i in range(0, len(alist), 4)]

            def emit_stage_group(b, grp, slots):
                # stg keeps the step-2 byte layout of the fp8 transpose
                # PSUM output; the copy runs as packed bf16 (DVE 2x mode)
                # and the PV lhsT reads fp8 at element step 2.
                stg = stgp.tile([P, 4, P, 2], F8, tag="stg")
                pt = psum_t.tile([P, 4, P, 2], F8, tag="pt")
                tr = []
                for k, a in enumerate(grp):
                    if a == b:
                        # even-b mixed pair: copy stored block (b,b)
                        nc.vector.tensor_copy(
                            stg[:, k, :, 0], e_block(b, b)
                        )
                    else:
                        nc.tensor.transpose(
                            pt[:, k, :, 0], e_block(b, a), ident
                        )
                        tr.append(k)
                if tr:
                    k0, k1 = tr[0], tr[-1]
                    nc.vector.tensor_copy(
                        stg[:, k0 : k1 + 1, :, :].bitcast(BF16),
                        pt[:, k0 : k1 + 1, :, :].bitcast(BF16),
                    )
                for k, a in enumerate(grp):
                    slots[a] = (stg, k)

            def emit_pv(b, slots, next_groups, next_slots):
                """PV for query block b; interleave next block's staging
                transposes between pairs to keep PE fed and give the DVE
                copies time to drain."""
                ng = list(next_groups)
                pv = psum_pv.tile([P, 1024], F32, tag="psPV")
                for t in range(NT // 2):
                    a0 = 2 * t
                    if a0 + 1 <= b:
                        lhsT = e_pair(t, b)
                    else:
                        stg, k = slots[a0]
                        stg1, k1 = slots[a0 + 1]
                        assert stg is stg1 and k1 == k + 1
                        lhsT = stg[:, k : k + 2, :, 0]
                    for s, w in PV_STRIPS:
                        nc.tensor.matmul(
                            pv[:, s : s + w],
                            lhsT=lhsT,
                            rhs=xn_aug[:, a0 : a0 + 2, s : s + w],
                            start=(t == 0),
                            stop=(t == NT // 2 - 1),
                            perf_mode=DR,
                        )
                    if t % 2 == 1 and ng:
                        emit_stage_group(b + 1, ng.pop(0), next_slots)
                while ng:
                    emit_stage_group(b + 1, ng.pop(0), next_slots)

                # blend: out = x*(0.5 + 0.5*inv) + num' * (0.5*inv)
                # where inv = 1/(1 + den'), den' = pv[:, 768]
                d1 = work.tile([P, 1], F32, tag="d1")
                nc.vector.tensor_scalar_add(d1, pv[:, D : D + 1], 1.0)
                inv2 = work.tile([P, 1], F32, tag="inv2")
                nc.vector.reciprocal(inv2, d1)
                invh = work.tile([P, 1], F32, tag="invh")
                nc.vector.tensor_scalar_mul(invh, inv2, 0.5)
                sself = work.tile([P, 1], F32, tag="sself")
                nc.vector.tensor_scalar_add(sself, invh, 0.5)
                t_t = work.tile([P, D], F32, tag="t")
                nc.scalar.mul(t_t, pv[:, 0:D], invh)
                ot = work.tile([P, D], BF16, tag="ot")
                nc.vector.scalar_tensor_tensor(
                    ot,
                    in0=x_full[:, b, :],
                    scalar=sself,
                    in1=t_t,
                    op0=mybir.AluOpType.mult,
                    op1=mybir.AluOpType.add,
                )
                nc.sync.dma_start(
                    out=out_dram[b * P : (b + 1) * P, :], in_=ot
                )

            slots = {}
            for grp in stage_groups(0):
                emit_stage_group(0, grp, slots)
            for b in range(NT):
                next_slots = {}
                ng = stage_groups(b + 1) if b + 1 < NT else []
                emit_pv(b, slots, ng, next_slots)
                slots = next_slots

    if not nc.is_finalized():
        nc.finalize()
    return nc


def _get_program():
    if "nc" not in _CACHED:
        _CACHED["nc"] = build_program()
    return _CACHED["nc"]


def kernel(**inputs):
    features = inputs["features"]
    assert features.shape == (B, H, W, D), features.shape
    x = np.ascontiguousarray(features.reshape(B, N, D)).astype(np.float32)
    nc = _get_program()
    in_maps = [{"x": x[b]} for b in range(B)]
    res = run_bass_kernel_spmd(nc, in_maps, core_ids=list(range(B)))
    out = np.stack(
        [np.asarray(res.results[b]["out"]).astype(np.float32) for b in range(B)],
        axis=0,
    )
    return out.reshape(B, H, W, D)


# revision 26
# speedup vs baseline: 1.0040x; 1.0040x over previous
"""CLIPtrase recalibration kernel for Trainium2 (Bass/Tile), fp8 DoubleRow.

Per-batch computation (B=8, one batch element per NeuronCore):
    x  : (2304, 768) f32
    xn = x / ||x||_row
    S  = xn @ xn.T              (cosine correlation, symmetric)
    W  = softmax(S / 0.05, axis=-1)
    out = 0.5 * x + 0.5 * (W @ x)

Key ideas vs the bf16 baseline:
  - All matmuls run in fp8e4 (e4m3) with MatmulPerfMode.DoubleRow: each
    instruction contracts TWO 128-deep k-tiles (157 TF/s peak, 2x bf16).
  - Only normalized xn is kept in fp8. PV needs E @ x = (n_a * E) @ xn, so
    ln(n_a) is folded into the exp bias (per-partition bias AP):
    E2^T[a,q] = exp(20*S^T + (ln n_a - 20)). The softmax denominator comes
    from an aug column of xn holding 1/n_a.
  - Exact self-term: the diagonal of S is killed in PSUM by accumulating a
    -240*I matmul before exp, so E2 has zero diagonal. The true diagonal
    weight is exp(0) = 1 exactly, re-added in f32 at blend time:
        recal = (x + num') / (1 + den')
    so fp8 quantization never touches the dominant self-term.
  - ACT uses only Ln/Exp/Copy — all in the natural_log_exp_and_others
    table, so the act table is loaded once (no ACT_TABLE_LOAD thrash).
    1/n = Exp(-0.5*Ln(ssum)); ln n - 20 = Copy(0.5*Ln(ssum) - 20).
  - Engine balance: ACT = exps + blend muls; DVE = ssum + PSUM->SBUF
    copies + blend scalars; GPSIMD = xn casts + blend fma (SBUF-only ops);
    PE is the critical path (~95us of fp8 matmul + transposes).
  - E^T blocks for the lower triangle are PE transposes of stored upper
    blocks (fp8 transpose writes PSUM at element step 2), staged to SBUF
    by DVE, interleaved between PV pairs to keep PE fed.
  - Output stored bf16 (halves output DMA); host casts back to f32.
"""

import sys

sys.path.insert(0, "/opt/trn_rl_repo")

import numpy as np

import concourse.bass as bass
import concourse.mybir as mybir
from concourse import bacc
from concourse.bass_utils import run_bass_kernel_spmd
from concourse.masks import make_identity
from concourse.tile import TileContext

F32 = mybir.dt.float32
BF16 = mybir.dt.bfloat16
F8 = mybir.dt.float8e4
DR = mybir.MatmulPerfMode.DoubleRow
AF = mybir.ActivationFunctionType

B = 8
H = 48
W = 48
N = H * W          # 2304
D = 768
P = 128
NT = N // P        # 18 row tiles
DT = D // P        # 6 feature tiles
DA = D + 4         # aug width: col 768 = 1/n, cols 769..771 = 0 pad
TEMP_INV = 20.0    # 1 / 0.05
# query megablocks: (start, width)
MEGAS = [(0, 512), (512, 512), (1024, 512), (1536, 512), (2048, 256)]
# PV strips within the psum tile; avoid crossing the 512-col bank boundary
PV_STRIPS = [(0, 256), (256, 256), (512, 256), (768, 4)]
NG = 4             # norm batch: tiles per DVE norm group (aligns megas)
# DVE bit-trick constants: ln n - 20 = 0.5*ln(ssum) - 20 via the float
# bit pattern (log2(s) ~ i/2^23 - 126.955), rsqrt via 0x5f3759df + Newton
LN_C1 = 4.1314791474339085e-08
LN_C2 = -63.99925015399393
RSQRT_MAGIC = 1597463007
I32 = mybir.dt.int32

_CACHED = {}


def build_program():
    nc = bacc.Bacc()
    x_in = nc.declare_dram_parameter("x", [N, D], F32, isOutput=False)
    out_dram = nc.declare_dram_parameter("out", [N, D], BF16, isOutput=True)

    mega_of_block = {}
    amax = []
    for mi, (q0, qw) in enumerate(MEGAS):
        for b in range(q0 // P, (q0 + qw) // P):
            mega_of_block[b] = mi
        amax.append((q0 + qw) // P - 1)

    with TileContext(nc) as tc:
        with (
            tc.tile_pool(name="persist", bufs=1) as persist,
            tc.tile_pool(name="work", bufs=3) as work,
            tc.tile_pool(name="stg", bufs=10) as stgp,
            tc.tile_pool(name="estore", bufs=1) as estore,
            tc.tile_pool(name="psS", bufs=2, space="PSUM") as psum_s,
            tc.tile_pool(name="psPV", bufs=2, space="PSUM") as psum_pv,
            tc.tile_pool(name="psT", bufs=2, space="PSUM") as psum_t,
        ):
            # persistent tensors
            x_full = persist.tile([P, NT, D], F32)     # raw x, resident
            xT = persist.tile([P, DT, N], F8)          # xn^T  [d, n]
            xn_aug = persist.tile([P, NT, DA], F8)     # [xn | 1/n | 0 0 0]

            # input DMAs issued before any engine init so transfers start
            # immediately; alternating issue queues halves the ~600ns
            # per-issue serialization
            for j in range(NT):
                eng = nc.sync if j % 2 == 0 else nc.gpsimd
                if j < 4:
                    # split early tiles so the first norm chain starts sooner
                    hd = D // 2
                    for c in range(2):
                        eng.dma_start(
                            out=x_full[:, j, c * hd : (c + 1) * hd],
                            in_=x_in[j * P : (j + 1) * P,
                                     c * hd : (c + 1) * hd],
                        )
                else:
                    eng.dma_start(
                        out=x_full[:, j, :],
                        in_=x_in[j * P : (j + 1) * P, :],
                    )

            ident = persist.tile([P, P], F8)
            make_identity(nc, ident)
            negdiag = persist.tile([P, P], F8)         # -240 * I
            nc.gpsimd.memset(negdiag, 0.0)
            nc.gpsimd.affine_select(
                out=negdiag,
                in_=negdiag,
                compare_op=mybir.AluOpType.not_equal,
                fill=-240.0,
                base=0,
                pattern=[[-1, P]],
                channel_multiplier=1,
            )
            nc.vector.memset(xn_aug[:, :, D + 1 : DA], 0.0)
            zero_bias = persist.tile([P, 1], F32)
            nc.vector.memset(zero_bias, 0.0)
            # dependency-free Exp so the ACT table load happens during
            # program init instead of after the first DMA wait
            dummy = persist.tile([P, 1], F32)
            nc.scalar.activation(dummy, zero_bias, AF.Exp, bias=zero_bias)
            lnb = persist.tile([P, NT], F32)           # ln(n) - 20 per tile
            invn = persist.tile([P, NT], F32)          # 1/n per tile
            ssums = persist.tile([P, NT], F32)         # sum(x^2) per tile

            # E2 storage: per mega m, E2^T[a, q-cols] for a <= amax(m)
            e_tiles = [
                estore.tile(
                    [P, amax[mi] + 1, qw], F8, tag=f"E{mi}", name=f"E{mi}"
                )
                for mi, (q0, qw) in enumerate(MEGAS)
            ]

            # ---- Phase 1+2 fused: load, norms, xn, xT; then per norm
            # group emit the QK mega whose tiles just completed ----
            def emit_norm_group(g0, g1):
                """Norms for tiles [g0, g1) entirely on DVE via float bit
                tricks (no ACT Ln/Sqrt -> single ACT table for the kernel).
                lnb = 0.5*ln(ssum) - 20 from the exponent bits;
                invn = rsqrt(ssum) via 0x5f3759df + one Newton step."""
                k = g1 - g0
                sl = slice(g0, g1)
                ibits = ssums[:, sl].bitcast(I32)
                fi = work.tile([P, k], F32, tag="nfi")
                nc.vector.tensor_copy(fi, ibits)  # int32 -> f32 numeric
                nc.vector.tensor_scalar(
                    lnb[:, sl], fi, LN_C1, LN_C2,
                    op0=mybir.AluOpType.mult, op1=mybir.AluOpType.add,
                )
                ish = work.tile([P, k], I32, tag="nish")
                nc.vector.tensor_scalar(
                    ish, ibits, 1, None,
                    op0=mybir.AluOpType.logical_shift_right,
                )
                iy = work.tile([P, k], I32, tag="niy")
                nc.vector.tensor_scalar(
                    iy, ish, -1, RSQRT_MAGIC,
                    op0=mybir.AluOpType.mult, op1=mybir.AluOpType.add,
                )
                y0 = iy.bitcast(F32)
                t1 = work.tile([P, k], F32, tag="nt1")
                nc.vector.tensor_mul(t1, y0, y0)
                t2 = work.tile([P, k], F32, tag="nt2")
                nc.vector.tensor_mul(t2, t1, ssums[:, sl])
                t3 = work.tile([P, k], F32, tag="nt3")
                nc.vector.tensor_scalar(
                    t3, t2, -0.5, 1.5,
                    op0=mybir.AluOpType.mult, op1=mybir.AluOpType.add,
                )
                nc.vector.tensor_mul(invn[:, sl], y0, t3)
                # aug cols: 1/n in fp8
                nc.vector.tensor_copy(xn_aug[:, sl, D : D + 1], invn[:, sl])

            def emit_tile(jj):
                # xn = x * (1/n), cast to fp8 (ACT Copy w/ scale AP)
                nc.scalar.mul(
                    xn_aug[:, jj, 0:D],
                    x_full[:, jj, :],
                    invn[:, jj : jj + 1],
                )
                # transpose xn into xT columns (fp8 transpose writes PSUM
                # at element step 2)
                ptj = psum_t.tile([P, DT, P, 2], F8, tag="pt")
                for d in range(DT):
                    nc.tensor.transpose(
                        ptj[:, d, :, 0],
                        xn_aug[:, jj, d * P : (d + 1) * P],
                        ident,
                    )
                nc.vector.tensor_copy(
                    xT[:, :, jj * P : (jj + 1) * P], ptj[:, :, :, 0]
                )

            def emit_qk_mega(mi):
                q0, qw = MEGAS[mi]
                et = e_tiles[mi]
                for a in range(amax[mi] + 1):
                    c0 = max(0, a * P - q0)
                    has_diag = a * P >= q0  # diag block at cols c0:c0+128
                    ps = psum_s.tile([P, 512], F32, tag="psS")
                    strips = []
                    s = c0
                    while s < qw:
                        w = min(256, qw - s)
                        strips.append((s, w))
                        s += w
                    for dp in range(DT // 2):
                        for si, (s, w) in enumerate(strips):
                            stop = dp == DT // 2 - 1 and not (
                                has_diag and si == 0
                            )
                            nc.tensor.matmul(
                                ps[:, s : s + w],
                                lhsT=xT[:, 2 * dp : 2 * dp + 2,
                                        a * P : (a + 1) * P],
                                rhs=xT[:, 2 * dp : 2 * dp + 2,
                                       q0 + s : q0 + s + w],
                                start=(dp == 0),
                                stop=stop,
                                perf_mode=DR,
                            )
                    if has_diag:
                        # kill diagonal: S += -240*I on cols c0:c0+128
                        nc.tensor.matmul(
                            ps[:, c0 : c0 + P],
                            lhsT=negdiag,
                            rhs=ident,
                            start=False,
                            stop=True,
                            skip_group_check=True,
                        )
                    # E2^T = exp(20*S + (ln n_a - 20)), fp8 out
                    nc.scalar.activation(
                        et[:, a, c0:qw],
                        ps[:, c0:qw],
                        AF.Exp,
                        bias=lnb[:, a : a + 1],
                        scale=TEMP_INV,
                    )

            # driver: norm groups align with megas (amax = 3,7,11,15,17);
            # mega mi is emitted as soon as its tiles complete. First two
            # groups are small to shorten the startup dependency chain.
            groups = [(0, 2, None), (2, 4, 0), (4, 8, 1), (8, 12, 2),
                      (12, 16, 3), (16, 18, 4)]
            for g0, g1, mi in groups:
                for j in range(g0, g1):
                    # sum of squares: ACT Square + accumulator
                    scratch = work.tile([P, D], BF16, tag="scratch")
                    nc.scalar.activation(
                        scratch,
                        x_full[:, j, :],
                        AF.Square,
                        bias=zero_bias,
                        accum_out=ssums[:, j : j + 1],
                    )
                emit_norm_group(g0, g1)
                for j in range(g0, g1):
                    emit_tile(j)
                if mi is not None:
                    emit_qk_mega(mi)

            def e_block(a, b):
                """AP of stored E2^T[a-tile, b-tile cols] (needs a <= b)."""
                mi = mega_of_block[b]
                q0 = MEGAS[mi][0]
                off = b * P - q0
                return e_tiles[mi][:, a, off : off + P]

            def e_pair(t, b):
                """AP of stored E2^T pair (2t,2t+1) x b-cols (2t+1 <= b)."""
                mi = mega_of_block[b]
                q0 = MEGAS[mi][0]
                off = b * P - q0
                return e_tiles[mi][:, 2 * t : 2 * t + 2, off : off + P]

            # ---- Phase 3: PV (fp8 DR) + staging + blend, pipelined ----
            def stage_groups(b):
                """Group list for b: each group is <=4 consecutive k-tiles
                to stage (transpose or copy) into one stg tile."""
                first = b if b % 2 == 0 else b + 1
                alist = list(range(first, NT))
                return [alist[i : i + 4] for i in range(0, len(alist), 4)]

            def emit_stage_group(b, grp, slots):
                # stg keeps the step-2 byte layout of the fp8 transpose
                # PSUM output; the copy runs as packed bf16 (DVE 2x mode)
                # and the PV lhsT reads fp8 at element step 2.
                stg = stgp.tile([P, 4, P, 2], F8, tag="stg")
                pt = psum_t.tile([P, 4, P, 2], F8, tag="pt")
                tr = []
                for k, a in enumerate(grp):
                    if a == b:
                        # even-b mixed pair: copy stored block (b,b)
                        nc.vector.tensor_copy(
                            stg[:, k, :, 0], e_block(b, b)
                        )
                    else:
                        nc.tensor.transpose(
                            pt[:, k, :, 0], e_block(b, a), ident
                        )
                        tr.append(k)
                if tr:
                    k0, k1 = tr[0], tr[-1]
                    nc.vector.tensor_copy(
                        stg[:, k0 : k1 + 1, :, :].bitcast(BF16),
                        pt[:, k0 : k1 + 1, :, :].bitcast(BF16),
                    )
                for k, a in enumerate(grp):
                    slots[a] = (stg, k)

            def emit_pv(b, slots, next_groups, next_slots):
                """PV for query block b; interleave next block's staging
                transposes between pairs to keep PE fed and give the DVE
                copies time to drain."""
                ng = list(next_groups)
                pv = psum_pv.tile([P, 1024], F32, tag="psPV")
                for t in range(NT // 2):
                    a0 = 2 * t
                    if a0 + 1 <= b:
                        lhsT = e_pair(t, b)
                    else:
                        stg, k = slots[a0]
                        stg1, k1 = slots[a0 + 1]
                        assert stg is stg1 and k1 == k + 1
                        lhsT = stg[:, k : k + 2, :, 0]
                    for s, w in PV_STRIPS:
                        nc.tensor.matmul(
                            pv[:, s : s + w],
                            lhsT=lhsT,
                            rhs=xn_aug[:, a0 : a0 + 2, s : s + w],
                            start=(t == 0),
                            stop=(t == NT // 2 - 1),
                            perf_mode=DR,
                        )
                    if t % 2 == 1 and ng:
                        emit_stage_group(b + 1, ng.pop(0), next_slots)
                while ng:
                    emit_stage_group(b + 1, ng.pop(0), next_slots)

                # blend: out = x*(0.5 + 0.5*inv) + num' * (0.5*inv)
                # where inv = 1/(1 + den'), den' = pv[:, 768]
                d1 = work.tile([P, 1], F32, tag="d1")
                nc.vector.tensor_scalar_add(d1, pv[:, D : D + 1], 1.0)
                inv2 = work.tile([P, 1], F32, tag="inv2")
                nc.vector.reciprocal(inv2, d1)
                invh = work.tile([P, 1], F32, tag="invh")
                nc.vector.tensor_scalar_mul(invh, inv2, 0.5)
                sself = work.tile([P, 1], F32, tag="sself")
                nc.vector.tensor_scalar_add(sself, invh, 0.5)
                t_t = work.tile([P, D], F32, tag="t")
                nc.scalar.mul(t_t, pv[:, 0:D], invh)
                ot = work.tile([P, D], BF16, tag="ot")
                nc.vector.scalar_tensor_tensor(
                    ot,
                    in0=x_full[:, b, :],
                    scalar=sself,
                    in1=t_t,
                    op0=mybir.AluOpType.mult,
                    op1=mybir.AluOpType.add,
                )
                nc.sync.dma_start(
                    out=out_dram[b * P : (b + 1) * P, :], in_=ot
                )

            slots = {}
            for grp in stage_groups(0):
                emit_stage_group(0, grp, slots)
            for b in range(NT):
                next_slots = {}
                ng = stage_groups(b + 1) if b + 1 < NT else []
                emit_pv(b, slots, ng, next_slots)
                slots = next_slots

    if not nc.is_finalized():
        nc.finalize()
    return nc


def _get_program():
    if "nc" not in _CACHED:
        _CACHED["nc"] = build_program()
    return _CACHED["nc"]


def kernel(**inputs):
    features = inputs["features"]
    assert features.shape == (B, H, W, D), features.shape
    x = np.ascontiguousarray(features.reshape(B, N, D)).astype(np.float32)
    nc = _get_program()
    in_maps = [{"x": x[b]} for b in range(B)]
    res = run_bass_kernel_spmd(nc, in_maps, core_ids=list(range(B)))
    out = np.stack(
        [np.asarray(res.results[b]["out"]).astype(np.float32) for b in range(B)],
        axis=0,
    )
    return out.reshape(B, H, W, D)
